# revision 14
# baseline (speedup 1.0000x reference)
"""Trainium2 Bass kernel for nn_MiddleLayerEncoder (gnn_message_passing).

Strategy: shard by CLUSTER across the 8 cores (each core owns 512 whole
clusters and all of their points), so both segment_max reductions are
core-local and no collectives are needed.  Host-side prep sorts points by
cluster, pads every cluster to a canonical per-rank size (identical across
cores -> single SPMD program), and bakes all segment boundaries into the
instruction stream.  Activations are kept transposed [feat, points] so the
per-cluster max is a free-axis windowed reduce; the neigh_enc[cluster]
gather is fused into the W1 matmul via a per-tile local one-hot operand.
"""

import numpy as np
import ml_dtypes
from contextlib import ExitStack

import concourse.bass as bass
import concourse.bacc as bacc
import concourse.tile as tile
from concourse import mybir
from concourse.bass_utils import run_bass_kernel_spmd

BF16 = mybir.dt.bfloat16
F32 = mybir.dt.float32
NPBF16 = ml_dtypes.bfloat16

N_CORES = 8
N_PTS = 262144
N_CLUSTERS = 4096
G = 24            # one-hot rows (max clusters overlapping a 512-col tile)
K1 = 3 + 64 + G   # layer-1 contraction: pts(3) + feat(64) + onehot(G)
MINL = 36         # minimum padded points per cluster (bounds clusters/tile)
CHUNK_COLS = 8192
TILE = 512


# ---------------------------------------------------------------- planning

def _plan(cluster):
    """Canonical SPMD layout shared by all cores."""
    counts = np.bincount(cluster, minlength=N_CLUSTERS)
    assert counts.min() >= 1, "empty cluster unsupported"
    order = np.argsort(-counts, kind="stable")  # cluster ids, size desc

    # snake-deal into N_CORES bins -> per-core 512 clusters, balanced sizes
    n_ranks = N_CLUSTERS // N_CORES
    cids = np.empty((N_CORES, n_ranks), dtype=np.int64)
    for i, cid in enumerate(order):
        rnd, pos = divmod(i, N_CORES)
        core = pos if rnd % 2 == 0 else N_CORES - 1 - pos
        cids[core, rnd] = cid

    sizes = counts[cids]                      # [cores, ranks]
    L = sizes.max(axis=0)                     # canonical per-rank size
    L = np.maximum((L + 3) // 4 * 4, MINL).astype(np.int64)

    col0 = np.concatenate([[0], np.cumsum(L)])  # rank -> start col
    S = int(col0[-1])

    # chunks: whole clusters, <= CHUNK_COLS cols, <= 128 clusters
    chunks = []  # (r0, r1, c0, cols)
    r0 = 0
    while r0 < n_ranks:
        r1 = r0
        while (
            r1 < n_ranks
            and (col0[r1 + 1] - col0[r0]) <= CHUNK_COLS
            and (r1 - r0) < 128
        ):
            r1 += 1
        chunks.append((r0, r1, int(col0[r0]), int(col0[r1] - col0[r0])))
        r0 = r1

    # rank of every column (canonical)
    col_rank = np.repeat(np.arange(n_ranks), L)

    # layer-1 512-col tiles per chunk: (c0, cols, base_rank, n_rank_rows)
    tiles = []
    for (r0, r1, c0, cc) in chunks:
        tl = []
        for t0 in range(0, cc, TILE):
            tc = min(TILE, cc - t0)
            base = int(col_rank[c0 + t0])
            last = int(col_rank[c0 + t0 + tc - 1])
            nrows = last - base + 1
            assert nrows <= G, f"tile spans {nrows} clusters > G={G}"
            tl.append((c0 + t0, tc, base, nrows))
        tiles.append(tl)

    # size classes (global, for stage-2 reduce): runs of equal L
    classes = []  # (rank0, n, w_points)
    i = 0
    while i < n_ranks:
        j = i
        while j < n_ranks and L[j] == L[i]:
            j += 1
        classes.append((i, j - i, int(L[i])))
        i = j

    return dict(
        cids=cids, sizes=sizes, L=L, col0=col0, S=S, chunks=chunks,
        col_rank=col_rank, tiles=tiles, classes=classes, n_ranks=n_ranks,
    )


def _prep_core(k, plan, rel_points, features, cluster, sort_idx, bucket0):
    """Per-core input arrays (canonical layout, core-specific data)."""
    L, col0, S = plan["L"], plan["col0"], plan["S"]
    cids = plan["cids"][k]
    n_ranks = plan["n_ranks"]

    slot = np.empty(S, dtype=np.int64)
    for r in range(n_ranks):
        cid = cids[r]
        idx = sort_idx[bucket0[cid]: bucket0[cid + 1]]
        n = idx.shape[0]
        c0, c1 = col0[r], col0[r + 1]
        m = c1 - c0
        slot[c0: c0 + n] = idx
        if m > n:
            slot[c0 + n: c1] = idx[0]

    pts = rel_points[slot]          # [S, 3] f32
    feat = features[slot]           # [S, 64] f32

    # encT: [K1, S] = ptsT(3) + featT(64) + onehot(G)
    encT = np.zeros((K1, S), dtype=NPBF16)
    encT[0:3] = pts.T.astype(NPBF16)
    encT[3:67] = feat.T.astype(NPBF16)
    col_rank = plan["col_rank"]
    oh_row = np.empty(S, dtype=np.int64)
    for tl in plan["tiles"]:
        for (c0, tc, base, nrows) in tl:
            oh_row[c0: c0 + tc] = col_rank[c0: c0 + tc] - base
    encT[67 + oh_row, np.arange(S)] = NPBF16(1.0)

    # pts4: [12, S/4] quad-packed points
    pts4 = (
        pts.astype(NPBF16)
        .reshape(S // 4, 4, 3)
        .transpose(1, 2, 0)
        .reshape(12, S // 4)
    )
    pts4 = np.ascontiguousarray(pts4)
    return {"encT": encT, "pts4": pts4}


def _blockdiag(w, times):
    fi, fo = w.shape
    out = np.zeros((fi * times, fo * times), dtype=w.dtype)
    for i in range(times):
        out[i * fi:(i + 1) * fi, i * fo:(i + 1) * fo] = w
    return out


def _prep_weights(inp):
    bf = lambda a: np.ascontiguousarray(a.astype(NPBF16))
    f32c = lambda a: np.ascontiguousarray(a.reshape(-1, 1).astype(np.float32))
    W1 = inp["W1"]
    return {
        "enc1_lhsT": bf(_blockdiag(inp["enc_W1"], 4)),       # [12,128]
        "b_enc1_4": f32c(np.tile(inp["enc_b1"], 4)),          # [128,1]
        "enc2_lhsT": bf(_blockdiag(inp["enc_W2"], 2)),        # [64,128]
        "b_enc2": f32c(inp["enc_b2"]),                        # [64,1]
        "W1ab": bf(W1[0:67]),                                 # [67,128]
        "W1c": bf(W1[67:131]),                                # [64,128]
        "b1": f32c(inp["b1"]),
        "fcW2": bf(inp["W2"]),                                # [128,128]
        "b2": f32c(inp["b2"]),
        "G1": bf(inp["G1"]),
        "gb1": f32c(inp["gb1"]),
        "G2a": bf(inp["G2"][:, 0:128]),
        "G2b": bf(inp["G2"][:, 128:256]),
        "gb2a": f32c(inp["gb2"][0:128]),
        "gb2b": f32c(inp["gb2"][128:256]),
    }


# ---------------------------------------------------------------- program

def _build(plan, reps=1):
    S = plan["S"]
    nc = bacc.Bacc(None, target_bir_lowering=False, debug=True)

    encT_d = nc.dram_tensor("encT", [K1, S], BF16, kind="ExternalInput")
    pts4_d = nc.dram_tensor("pts4", [12, S // 4], BF16, kind="ExternalInput")
    wspec = [
        ("enc1_lhsT", [12, 128], BF16), ("b_enc1_4", [128, 1], F32),
        ("enc2_lhsT", [64, 128], BF16), ("b_enc2", [64, 1], F32),
        ("W1ab", [67, 128], BF16), ("W1c", [64, 128], BF16),
        ("b1", [128, 1], F32), ("fcW2", [128, 128], BF16),
        ("b2", [128, 1], F32), ("G1", [128, 128], BF16),
        ("gb1", [128, 1], F32), ("G2a", [128, 128], BF16),
        ("G2b", [128, 128], BF16), ("gb2a", [128, 1], F32),
        ("gb2b", [128, 1], F32),
    ]
    w_d = {n: nc.dram_tensor(n, sh, dt, kind="ExternalInput") for n, sh, dt in wspec}
    out_d = nc.dram_tensor("out", [256, 512], F32, kind="ExternalOutput")

    RELU = mybir.ActivationFunctionType.Relu
    COPY = mybir.ActivationFunctionType.Copy

    with tile.TileContext(nc) as tc, ExitStack() as ctx:
        consts = ctx.enter_context(tc.tile_pool(name="consts", bufs=1))
        glob = ctx.enter_context(tc.tile_pool(name="glob", bufs=1))
        stream = ctx.enter_context(tc.tile_pool(name="stream", bufs=2))
        small = ctx.enter_context(tc.tile_pool(name="small", bufs=3))
        ps_a = ctx.enter_context(tc.tile_pool(name="ps_a", bufs=2, space="PSUM"))
        ps_p = ctx.enter_context(tc.tile_pool(name="ps_p", bufs=3, space="PSUM"))

        w_sb = {}
        for n, sh, dt in wspec:
            t = consts.tile(sh, dt, tag=f"w_{n}")
            nc.sync.dma_start(out=t[:], in_=w_d[n][:])
            w_sb[n] = t
        # copy of enc2 weights staged at partitions 64-127 (matmul requires
        # lhsT and rhs to share base_partition; the B-half rhs lives there)
        enc2_hi = consts.tile([128, 128], BF16, tag="w_enc2_hi")
        nc.sync.dma_start(out=enc2_hi[64:128, :], in_=w_d["enc2_lhsT"][:])

        Cbuf = glob.tile([128, S // 4], BF16, tag="Cbuf")
        Dbuf = glob.tile([128, S // 2], BF16, tag="Dbuf")
        pre_neigh = glob.tile([128, plan["n_ranks"]], BF16, tag="pre_neigh")
        neighT = glob.tile([64, plan["n_ranks"]], BF16, tag="neighT")
        gT = glob.tile([128, plan["n_ranks"]], BF16, tag="gT")
        T2buf = glob.tile([128, plan["n_ranks"]], BF16, tag="T2buf")

        n_chunks = len(plan["chunks"])
        ST = 1024  # pair-stage super-tile width (2 PSUM banks)

        def enc_stage(k):
            (r0, r1, c0, cc) = plan["chunks"][k]
            q0, qc = c0 // 4, cc // 4
            pts4_t = stream.tile([12, qc], BF16, tag="pts4_t")
            nc.sync.dma_start(out=pts4_t[:], in_=pts4_d[:, q0:q0 + qc])
            encT_t = stream.tile([K1, cc], BF16, tag="encT_t")
            nc.sync.dma_start(out=encT_t[:], in_=encT_d[:, c0:c0 + cc])
            h1_t = stream.tile([128, qc], BF16, tag="h1_t")
            for s0 in range(0, qc, ST):
                sc = min(ST, qc - s0)
                # enc1 + h1 evac in 512-wide sub-tiles
                for t0 in range(s0, s0 + sc, TILE):
                    tcn = min(TILE, s0 + sc - t0)
                    p1 = ps_a.tile([128, TILE], F32, tag="psa")
                    nc.tensor.matmul(p1[:, :tcn], w_sb["enc1_lhsT"][:],
                                     pts4_t[:, t0:t0 + tcn], start=True, stop=True)
                    nc.scalar.activation(h1_t[:, t0:t0 + tcn], p1[:, :tcn], RELU,
                                         bias=w_sb["b_enc1_4"][:], scale=1.0)
                # enc2 pair over the super tile
                pA = ps_p.tile([128, ST], F32, tag="psp")
                pB = ps_p.tile([128, ST], F32, tag="psp")
                for t0 in range(0, sc, TILE):
                    tcn = min(TILE, sc - t0)
                    sl = slice(s0 + t0, s0 + t0 + tcn)
                    nc.tensor.matmul(pA[:, t0:t0 + tcn], w_sb["enc2_lhsT"][:],
                                     h1_t[0:64, sl], start=True, stop=True)
                    nc.tensor.matmul(pB[:, t0:t0 + tcn], enc2_hi[64:128, :],
                                     h1_t[64:128, sl], start=True, stop=True)
                A_sb = small.tile([128, ST], BF16, tag="A1sb")
                nc.scalar.activation(A_sb[:, :sc], pA[:, :sc], COPY)
                nc.vector.tensor_max(Cbuf[:, q0 + s0: q0 + s0 + sc],
                                     pB[:, :sc], A_sb[:, :sc])
            return encT_t  # consumed later by l_stage(k)

        def seg1_and_M(k):
            (r0, r1, c0, cc) = plan["chunks"][k]
            q0 = c0 // 4
            nk = r1 - r0
            # per-class windowed reduce within this chunk
            i = r0
            while i < r1:
                j = i
                w = int(plan["L"][i])
                while j < r1 and plan["L"][j] == w:
                    j += 1
                n, wq = j - i, w // 4
                o = q0 + (int(plan["col0"][i]) - c0) // 4
                src = Cbuf[:, o: o + n * wq].rearrange("p (n w) -> p n w", w=wq)
                nc.vector.reduce_max(pre_neigh[:, i:j], src,
                                     axis=mybir.AxisListType.X)
                i = j
            fold = small.tile([64, 128], BF16, tag="fold")
            nc.sync.dma_start(out=fold[:, :nk], in_=pre_neigh[64:128, r0:r1])
            mx = small.tile([64, 128], BF16, tag="mx")
            nc.vector.tensor_max(mx[:, :nk], pre_neigh[0:64, r0:r1], fold[:, :nk])
            nc.scalar.activation(neighT[:, r0:r1], mx[:, :nk], RELU,
                                 bias=w_sb["b_enc2"][:], scale=1.0)
            pm = ps_a.tile([128, 128], F32, tag="psa")
            nc.tensor.matmul(pm[:nk, :], neighT[:, r0:r1], w_sb["W1c"][:],
                             start=True, stop=True)
            M_chunk = small.tile([128, 128], BF16, tag="Mchunk")
            if nk < 128:
                nc.vector.memset(M_chunk[:], 0.0)  # zero first, then fill
            nc.scalar.activation(M_chunk[:nk, :], pm[:nk, :], COPY)
            return M_chunk

        def l_stage(k, M_chunk):
            (r0, r1, c0, cc) = plan["chunks"][k]
            encT_t = enc_tiles[k]
            e1_t = stream.tile([128, cc], BF16, tag="e1_t")
            e1_v = e1_t[:].rearrange("p (h i) -> p i h", h=2)
            for ti, (tc0, tcn, base, nrows) in enumerate(plan["tiles"][k]):
                lt = small.tile([128, 128], BF16, tag="lhsT1")
                # always copy G rows (zeros beyond this chunk's clusters come
                # from M_chunk's zeroed tail); clamp at partition 128
                gr = min(G, 128 - (base - r0))
                if gr < G:
                    nc.vector.memset(lt[64:128, :], 0.0)  # aligned, pre-fill
                nc.sync.dma_start(out=lt[0:67, :], in_=w_sb["W1ab"][:])
                nc.sync.dma_start(out=lt[67:67 + gr, :],
                                  in_=M_chunk[base - r0: base - r0 + gr, :])
                p = ps_a.tile([128, TILE], F32, tag="psa")
                loc = tc0 - c0
                nc.tensor.matmul(p[:, :tcn], lt[0:K1, :],
                                 encT_t[:, loc: loc + tcn], start=True, stop=True)
                dst = e1_v[:, loc // 2: (loc + tcn) // 2, :]
                src = p[:, :tcn].rearrange("p (i h) -> p i h", h=2)
                if ti % 2 == 0:
                    nc.scalar.activation(dst, src, RELU, bias=w_sb["b1"][:],
                                         scale=1.0)
                else:
                    nc.vector.tensor_scalar(
                        dst, src, w_sb["b1"][:], 0.0,
                        op0=mybir.AluOpType.add, op1=mybir.AluOpType.max,
                    )
            # layer 2 + stage-2 L1 pairing (1024-wide super tiles)
            d0 = c0 // 2
            half = cc // 2
            for s0 in range(0, half, ST):
                sc = min(ST, half - s0)
                pA = ps_p.tile([128, ST], F32, tag="psp")
                pB = ps_p.tile([128, ST], F32, tag="psp")
                for t0 in range(0, sc, TILE):
                    tcn = min(TILE, sc - t0)
                    nc.tensor.matmul(pA[:, t0:t0 + tcn], w_sb["fcW2"][:],
                                     e1_t[:, s0 + t0: s0 + t0 + tcn],
                                     start=True, stop=True)
                    nc.tensor.matmul(pB[:, t0:t0 + tcn], w_sb["fcW2"][:],
                                     e1_t[:, half + s0 + t0: half + s0 + t0 + tcn],
                                     start=True, stop=True)
                A_sb = small.tile([128, ST], BF16, tag="A2sb")
                nc.scalar.activation(A_sb[:, :sc], pA[:, :sc], COPY)
                nc.vector.tensor_max(Dbuf[:, d0 + s0: d0 + s0 + sc],
                                     pB[:, :sc], A_sb[:, :sc])
            # stage-2 per-class reduce for this chunk (overlaps next chunks)
            T2 = T2buf
            i = r0
            while i < r1:
                j = i
                w = int(plan["L"][i])
                while j < r1 and plan["L"][j] == w:
                    j += 1
                n, wd = j - i, w // 2
                o = int(plan["col0"][i]) // 2
                src = Dbuf[:, o: o + n * wd].rearrange("p (n w) -> p n w", w=wd)
                nc.vector.reduce_max(T2[:, i:j], src, axis=mybir.AxisListType.X)
                i = j

        for rep in range(reps):
            # software-pipelined emission: enc(k+1) interleaves with l(k)
            enc_tiles = {}
            enc_tiles[0] = enc_stage(0)
            M_prev = seg1_and_M(0)
            for k in range(n_chunks):
                if k + 1 < n_chunks:
                    enc_tiles[k + 1] = enc_stage(k + 1)
                l_stage(k, M_prev)
                del enc_tiles[k]
                if k + 1 < n_chunks:
                    M_prev = seg1_and_M(k + 1)

            nc.scalar.activation(gT[:], T2buf[:], RELU, bias=w_sb["b2"][:],
                                 scale=1.0)

            # global MLP
            pg = ps_a.tile([128, 512], F32, tag="psa")
            nc.tensor.matmul(pg[:], w_sb["G1"][:], gT[:], start=True, stop=True)
            g1T = glob.tile([128, 512], BF16, tag="g1T")
            nc.scalar.activation(g1T[:], pg[:], RELU, bias=w_sb["gb1"][:],
                                 scale=1.0)
            for half, (wn, bn) in enumerate((("G2a", "gb2a"), ("G2b", "gb2b"))):
                po = ps_a.tile([128, 512], F32, tag="psa")
                nc.tensor.matmul(po[:], w_sb[wn][:], g1T[:], start=True,
                                 stop=True)
                o_sb = glob.tile([128, 512], F32, tag=f"osb{half}")
                nc.scalar.activation(o_sb[:], po[:], RELU, bias=w_sb[bn][:],
                                     scale=1.0)
                nc.sync.dma_start(out=out_d[half * 128:(half + 1) * 128, :],
                                  in_=o_sb[:])

    nc.finalize()
    return nc


# ---------------------------------------------------------------- entry

_CACHE = {}


def _run(inputs, trace=False, **spmd_kwargs):
    cluster = np.asarray(inputs["cluster"])
    key = hash(cluster.tobytes())
    if key not in _CACHE:
        plan = _plan(cluster)
        nc = _build(plan)
        _CACHE[key] = (plan, nc)
    plan, nc = _CACHE[key]

    rel_points = np.asarray(inputs["relative_points"], dtype=np.float32)
    features = np.asarray(inputs["features"], dtype=np.float32)
    sort_idx = np.argsort(cluster, kind="stable")
    bucket0 = np.concatenate(
        [[0], np.cumsum(np.bincount(cluster, minlength=N_CLUSTERS))]
    )
    wmap = _prep_weights({k: np.asarray(v, dtype=np.float32)
                          for k, v in inputs.items()
                          if k not in ("relative_points", "features", "cluster")})

    in_maps = []
    for k in range(N_CORES):
        m = _prep_core(k, plan, rel_points, features, cluster, sort_idx, bucket0)
        m.update(wmap)
        in_maps.append(m)

    res = run_bass_kernel_spmd(nc, in_maps, list(range(N_CORES)),
                               trace=trace, **spmd_kwargs)

    out = np.empty((N_CLUSTERS, 256), dtype=np.float32)
    for k in range(N_CORES):
        out[plan["cids"][k]] = res.results[k]["out"].T
    return out, res


def kernel(**inputs):
    return _run(inputs)[0]
